# revision 35
# baseline (speedup 1.0000x reference)
"""DCGRU cell Trainium2 kernel: batch-parallel SPMD over 8 NeuronCores.

Sharding: data-parallel over batch B=16 -> 2 batches/core; supports and
weights replicated. No collectives.

The 4 diffusion product streams (A1/A2 for gconv1, B1/B2 for gconv2)
dominate: each streams S^T once. They run in fp8e4m3 with DoubleRow
perf mode (contraction of two 128-node blocks per matmul), halving both
HBM traffic (64MB/support/stream) and PE time vs bf16. S is scaled by
2^12 before fp8 quantization (its values ~2^-12 would flush to zero);
hop-1 outputs are re-quantized to fp8 with a 2^6 scale. Dense phases
(D1 ru/sigmoid, D2 c/tanh/blend) stay bf16.

Orientation: stationary lhsT = x[m-dblock, d] fp8 natural layout,
moving rhs = S^T[m-dblock, n_cols] fp8, psum = (S@x)^T [d, n] f32.

Per-core phases:
  0:  XT = x^T via DMA-transpose; X8 = fp8 x natural (host-cast)
  A1: x1_s^T = (S_s@x)^T        -> h1 DRAM bf16 + X1q natural fp8
  A2: x2_s^T = 2(S_s@x1)^T - x^T -> h1
  D1: ru^T = sigmoid(W_ru^T h^T + b); rs^T; XC2 = rs natural fp8 packed
  B1: x1'_s^T = (S_s@rs)^T packed -> h2 + XC3 natural fp8
  B2: x2'_s^T = 2(S_s@x1')^T - rs^T -> h2
  D2: c^T = tanh(W_c^T h'^T + b_c), out^T = c + u*(s - c), PE-transpose,
      DMA out. (inputs-half feats of gconv2 reuse gconv1's h1 rows 0:64)
"""

import sys

sys.path.insert(0, "/opt/trn_rl_repo")

from contextlib import ExitStack

import ml_dtypes
import numpy as np

import concourse.bacc as bacc
import concourse.bass as bass
import concourse.mybir as mybir
import concourse.tile as tile
from concourse.bass_utils import run_bass_kernel_spmd

BF16 = mybir.dt.bfloat16
F32 = mybir.dt.float32
F8 = mybir.dt.float8e4
AF = mybir.ActivationFunctionType
ALU = mybir.AluOpType
DR = mybir.MatmulPerfMode.DoubleRow

N = 8192
DC = 128          # D_IN + D_H
H = 64
B2 = 2            # batches per core
NBLK = N // 128   # 64 m-blocks
ND = NBLK // 2    # 32 double m-blocks (DoubleRow)
CH = 512          # psum chunk (free dim)
NCH = N // CH     # 16 chunks
# groups of chunks sharing one stationary load; 4 product psum banks
GCN = 2           # chunks per group
GC = GCN * CH     # group columns (1024)
NG = NCH // GCN   # 8 groups
OCT = 8           # double m-blocks per stream DMA (8KB contiguous runs)
NSUP = 2

S_SC = 2.0 ** 12   # host scale on S before fp8 quantization
X_SC = 2.0 ** 6    # scale on hop-1 outputs for fp8 re-quantization

_CACHE = {}


def _build():
    import os
    PHASES = int(os.environ.get("DCGRU_PHASES", "6"))
    nc = bacc.Bacc("TRN2", target_bir_lowering=False, debug=False)

    xc_d = nc.dram_tensor("xcat", [B2, N, DC], BF16, kind="ExternalInput")
    xc8_d = nc.dram_tensor("xcat8", [B2, N, DC], F8, kind="ExternalInput")
    # blocked S^T fp8: [s, group, k, p, a2, c]; per (g,k,p) the (a2,c)
    # range is contiguous, giving 4KB DMA descriptor runs
    sup_d = nc.dram_tensor("supB", [NSUP, NG, 2, 128, ND, GC], F8,
                           kind="ExternalInput")
    wru_d = nc.dram_tensor("wru", [5 * DC, 2 * H], BF16, kind="ExternalInput")
    wc_d = nc.dram_tensor("wc", [5 * DC, H], BF16, kind="ExternalInput")
    bru_d = nc.dram_tensor("bru", [2 * H, 1], F32, kind="ExternalInput")
    bc_d = nc.dram_tensor("bc", [H, 1], F32, kind="ExternalInput")
    out_d = nc.dram_tensor("out", [B2, N, H], BF16, kind="ExternalOutput")

    id_bf = nc.inline_tensor(np.eye(128, dtype=ml_dtypes.bfloat16), "id_bf")
    id_f = nc.inline_tensor(np.eye(128, dtype=np.float32), "id_f")
    id_8 = nc.inline_tensor(np.eye(128, dtype=ml_dtypes.float8_e4m3), "id_8")

    xc_ap = xc_d.ap()
    xc8_ap = xc8_d.ap()
    sup_ap = sup_d.ap()
    out_ap = out_d.ap()

    with tile.TileContext(nc) as tc, ExitStack() as ctx:
        cpool = ctx.enter_context(tc.tile_pool(name="const", bufs=1))
        dram = ctx.enter_context(tc.tile_pool(name="dram", bufs=1, space="DRAM"))
        pers = ctx.enter_context(tc.tile_pool(name="pers", bufs=1))
        st = ctx.enter_context(tc.tile_pool(name="st", bufs=2))
        stage = ctx.enter_context(tc.tile_pool(name="stage", bufs=6))
        onat = ctx.enter_context(tc.tile_pool(name="onat", bufs=3))
        pp = ctx.enter_context(tc.tile_pool(name="pp", bufs=4, space="PSUM"))
        pt = ctx.enter_context(tc.tile_pool(name="pt", bufs=4, space="PSUM"))

        # ---- constants: only ID8 loads up front (needed by A1's deferred
        # transposes); the rest queue after A1's stream emission so the
        # first stream DMAs aren't delayed behind them ----
        IDB = cpool.tile([128, 128], BF16, tag="idb", name="idb")
        IDF = cpool.tile([128, 128], F32, tag="idf", name="idf")
        ID8 = cpool.tile([128, 128], F8, tag="id8", name="id8")
        nc.sync.dma_start(ID8[:], id_8.ap())
        WRU = cpool.tile([128, 5 * 128], BF16, tag="wru", name="wru")
        # WC layout: cols m*64:(m+1)*64 = inputs-half block (rows 0:64);
        # cols 320+m*64 = states-half block, duplicated at rows 0:64 and 64:128
        WC = cpool.tile([128, 10 * 64], BF16, tag="wc", name="wc")
        BRU = cpool.tile([128, 1], F32, tag="bru", name="bru")
        BC = cpool.tile([64, 1], F32, tag="bc", name="bc")

        def load_late_consts():
            nc.sync.dma_start(IDB[:], id_bf.ap())
            nc.sync.dma_start(IDF[:], id_f.ap())
            nc.sync.dma_start(
                WRU[:].rearrange("p (a o) -> p a o", a=5),
                wru_d.ap().rearrange("(a p) o -> p a o", p=128),
            )
            for m in range(5):
                nc.sync.dma_start(
                    WC[0:64, m * 64:(m + 1) * 64],
                    wc_d.ap()[m * 128:m * 128 + 64, :]
                )
                nc.sync.dma_start(
                    WC[0:64, 320 + m * 64:320 + (m + 1) * 64],
                    wc_d.ap()[m * 128 + 64:(m + 1) * 128, :],
                )
                nc.sync.dma_start(
                    WC[64:128, 320 + m * 64:320 + (m + 1) * 64],
                    wc_d.ap()[m * 128 + 64:(m + 1) * 128, :],
                )
            nc.sync.dma_start(BRU[:], bru_d.ap())
            nc.sync.dma_start(BC[:], bc_d.ap())
            for b in range(B2):
                nc.sync.dma_start_transpose(XT[b][:], xc_ap[b])

        # ---- DRAM scratch: gconv1 product feats^T, i = 2s+hop:
        # (x1_s0, x2_s0, x1_s1, x2_s1) stacked for batched D-phase loads
        h1 = [dram.tile([4, 128, N], BF16, tag=f"h1_{b}", name=f"h1_{b}")
              for b in range(B2)]
        # gconv2 states-half feats^T, batch-packed rows (b*64), i = 2s+hop
        h2 = dram.tile([4, 128, N], BF16, tag="h2", name="h2")

        # ---- persistent SBUF ----
        XT = [pers.tile([128, N], BF16, tag="xt", name=f"XT_{b}", bufs=2)
              for b in range(B2)]
        X8 = [pers.tile([128, N], F8, tag="x8", name=f"X8_{b}", bufs=2)
              for b in range(B2)]
        X1q = [[pers.tile([128, N], F8, tag="x1q", name=f"X1q_{s}_{b}", bufs=4)
                for b in range(B2)] for s in range(NSUP)]

        # ---- phase 0: x natural fp8 from host (x^T DMA-transposes are in
        # load_late_consts, emitted after A1) ----
        for b in range(B2):
            nc.sync.dma_start(
                X8[b][:].rearrange("p (a d) -> p a d", a=NBLK),
                xc8_ap[b].rearrange("(a p) d -> p a d", p=128),
            )

        def dr_slice(T, a2):
            """[128, 2, 128] DoubleRow lhsT view of natural-layout tile T."""
            return T[:, a2 * 256:(a2 + 1) * 256].rearrange(
                "p (k d) -> p k d", k=2)

        def product_stream(lhs_of, psum_sink, pack_batches):
            """Stream supT once (fp8, DoubleRow over double m-blocks).

            lhs_of(s, b, a2) -> lhsT AP [128, 2, 128]. psum_sink(s,
            b_or_None, j, c0, cnt, psum) consumes the finished [128, CH]
            f32 psum for chunk c0+j and returns a deferred closure (PE
            transpose tail) or None. Deferred work is emitted after the
            NEXT group's matmuls so the PE never waits on the ACT/DVE
            psum-drain chain.
            """
            # supports interleaved per group so both supports' outputs for a
            # given column range complete early -> downstream dense phases
            # (D1/D2) overlap the stream instead of waiting for its end
            pending = []
            for g in range(NG):
                for s in range(NSUP):
                    if pack_batches:
                        psums = [pp.tile([128, CH], F32, tag="pp", name="pp")
                                 for j in range(GCN)]
                    else:
                        psums = [pp.tile([128, CH], F32, tag="pp", name="pp")
                                 for _ in range(B2 * GCN)]
                    for o in range(ND // OCT):
                        stt = st.tile([128, 2 * OCT * GC], F8, tag="st",
                                      name="st")
                        st4 = stt[:].rearrange("p (k a c) -> p k a c",
                                               k=2, a=OCT)
                        nc.sync.dma_start(
                            st4,
                            sup_ap[s, g, :, :, o * OCT:(o + 1) * OCT,
                                   :].rearrange("k p a c -> p k a c"),
                        )
                        for a in range(OCT):
                            a2 = o * OCT + a
                            first = a2 == 0
                            last = a2 == ND - 1
                            if pack_batches:
                                lhsT = lhs_of(s, None, a2)
                                for j in range(GCN):
                                    nc.tensor.matmul(
                                        psums[j][:], lhsT,
                                        st4[:, :, a, j * CH:(j + 1) * CH],
                                        start=first, stop=last, perf_mode=DR,
                                    )
                            else:
                                for b in range(B2):
                                    lhsT = lhs_of(s, b, a2)
                                    for j in range(GCN):
                                        nc.tensor.matmul(
                                            psums[b * GCN + j][:], lhsT,
                                            st4[:, :, a, j * CH:(j + 1) * CH],
                                            start=first, stop=last,
                                            perf_mode=DR,
                                        )
                    # previous group's transpose tails land after this
                    # group's matmuls in the PE queue
                    for fn in pending:
                        fn()
                    pending = []
                    if pack_batches:
                        for j in range(GCN):
                            d = psum_sink(s, None, g * GCN + j, psums[j])
                            if d:
                                pending.append(d)
                    else:
                        for b in range(B2):
                            for j in range(GCN):
                                d = psum_sink(s, b, g * GCN + j,
                                              psums[b * GCN + j])
                                if d:
                                    pending.append(d)
            for fn in pending:
                fn()

        def hop1_sink(h_dst, q_dst):
            """psum = 2^12 (S@x): h_dst gets unscaled bf16 ^T feats, q_dst
            gets fp8 2^6-scaled natural layout via PE transposes."""
            def sink(s, b, cc, psum):
                cols = slice(cc * CH, (cc + 1) * CH)
                t = stage.tile([128, CH], BF16, tag="sg", name="sg")
                nc.scalar.activation(t[:], psum[:], AF.Copy, scale=1.0 / S_SC)
                nc.sync.dma_start(h_dst(s, b)[:, cols], t[:])
                t8 = stage.tile([128, CH], F8, tag="s8", name="s8", bufs=8)
                nc.vector.tensor_scalar_mul(t8[:], psum[:], X_SC / S_SC)

                def deferred():
                    for tp in range(4):
                        blk = cc * 4 + tp
                        ps8 = pt.tile([128, 256], F8, tag="tp", name="tp")
                        ps8_s = ps8[:].rearrange("p (c two) -> p c two", two=2)[:, :, 0]
                        nc.tensor.transpose(
                            ps8_s, t8[:, tp * 128:(tp + 1) * 128], ID8[:]
                        )
                        nc.vector.tensor_copy(
                            q_dst(s, b)[:, blk * 128:(blk + 1) * 128], ps8_s
                        )
                return deferred
            return sink

        def hop2_sink(h_dst, sub_of):
            """psum = 2^18 (S@x1): x2 = psum*2^-17 - sub (bf16 ^T)."""
            def sink(s, b, cc, psum):
                cols = slice(cc * CH, (cc + 1) * CH)
                t = stage.tile([128, CH], BF16, tag="sg", name="sg")
                nc.vector.scalar_tensor_tensor(
                    t[:], psum[:], 2.0 / (S_SC * X_SC), sub_of(b)[:, cols],
                    op0=ALU.mult, op1=ALU.subtract,
                )
                nc.sync.dma_start(h_dst(s, b)[:, cols], t[:])
                return None
            return sink

        # ---- A1: x1_s^T = (S_s @ x)^T ----
        product_stream(
            lambda s, b, a2: dr_slice(X8[b], a2),
            hop1_sink(lambda s, b: h1[b][2 * s], lambda s, b: X1q[s][b]),
            pack_batches=False,
        )
        load_late_consts()

        if PHASES < 2:
            return nc
        # ---- A2: x2_s^T = 2*(S_s @ x1_s)^T - x^T ----
        product_stream(
            lambda s, b, a2: dr_slice(X1q[s][b], a2),
            hop2_sink(lambda s, b: h1[b][2 * s + 1], lambda b: XT[b]),
            pack_batches=False,
        )

        if PHASES < 3:
            return nc
        # ---- D1: dense ru + sigmoid + rs^T + XC2 natural fp8 ----
        RUT = [pers.tile([128, N], BF16, tag="big2", name=f"RUT_{b}", bufs=2)
               for b in range(B2)]
        RST = pers.tile([128, N], BF16, tag="rst", name="RST")
        # XC2/XC3 reuse X8's two fp8 slots (X8 is dead after A1)
        XC2 = pers.tile([128, N], F8, tag="x8", name="XC2", bufs=2)
        def d1_chunk(b, cc):
            cols = slice(cc * CH, (cc + 1) * CH)
            ps = pp.tile([128, CH], F32, tag="pp", name="pp")
            # one batched load of all 4 product feats (reuses x1q slots)
            sg4 = pers.tile([128, 4 * CH], BF16, tag="x1q", name="sg4",
                            bufs=4)
            nc.sync.dma_start(
                sg4[:].rearrange("p (i c) -> p i c", i=4),
                h1[b][:, :, cols].rearrange("i p c -> p i c"),
            )
            for i in range(5):
                if i == 0:
                    rhs = XT[b][:, cols]
                else:
                    rhs = sg4[:, (i - 1) * CH:i * CH]
                nc.tensor.matmul(
                    ps[:], WRU[:, i * 128:(i + 1) * 128], rhs,
                    start=(i == 0), stop=(i == 4),
                )
            nc.scalar.activation(
                RUT[b][:, cols], ps[:], AF.Sigmoid, bias=BRU[:]
            )
            # rs = r * states^T; base-shift states^T and the result via
            # single-input copies (SB-SB two-input ops need equal bases)
            sts = stage.tile([64, CH], BF16, tag="sh1", name="sh1", bufs=3)
            nc.vector.tensor_copy(sts[:], XT[b][64:128, cols])
            rsc = stage.tile([64, CH], BF16, tag="sh2", name="sh2", bufs=3)
            nc.vector.tensor_mul(rsc[:], RUT[b][0:64, cols], sts[:])
            nc.vector.tensor_copy(RST[b * 64:(b + 1) * 64, cols], rsc[:])

            def deferred():
                for tp in range(4):
                    blk = cc * 4 + tp
                    ps2 = pt.tile([128, 128], BF16, tag="tp", name="tp")
                    nc.tensor.transpose(
                        ps2[0:128, 0:64],
                        RST[b * 64:(b + 1) * 64, blk * 128:(blk + 1) * 128],
                        IDB[b * 64:(b + 1) * 64, b * 64:b * 64 + 64],
                    )
                    nc.vector.tensor_copy(
                        XC2[:, blk * 128 + b * 64:blk * 128 + b * 64 + 64],
                        ps2[0:128, 0:64],
                    )
            return deferred

        pend = []
        for b in range(B2):
            for cc in range(NCH):
                pend.append(d1_chunk(b, cc))
                if len(pend) > 3:
                    pend.pop(0)()
        for fn in pend:
            fn()

        if PHASES < 4:
            return nc
        # ---- B1: x1'_s^T packed = (S_s @ rs)^T ----
        XC3 = pers.tile([128, N], F8, tag="x8", name="XC3", bufs=2)
        product_stream(
            lambda s, b, a2: dr_slice(XC2, a2),
            hop1_sink(lambda s, b: h2[2 * s], lambda s, b: XC3),
            pack_batches=True,
        )

        if PHASES < 5:
            return nc
        # ---- B2: x2'_s^T packed = 2*(S_s @ x1')^T - rs^T ----
        product_stream(
            lambda s, b, a2: dr_slice(XC3, a2),
            hop2_sink(lambda s, b: h2[2 * s + 1], lambda b: RST),
            pack_batches=True,
        )

        if PHASES < 6:
            return nc
        # ---- D2: dense c + tanh + blend + transpose + out ----
        # blend runs at base-64 partitions so the two-input DVE ops read
        # XT/RUT rows 64:128 directly (no alignment copies)
        def d2_chunk(b, cc):
            cols = slice(cc * CH, (cc + 1) * CH)
            ps = pp.tile([128, CH], F32, tag="pp", name="pp")
            pc = ps[0:64, :]
            # batched loads: 4 inputs-half feats (h1 rows 0:64) and 4
            # states-half feats (h2 rows b*64:), reusing x1q slots
            sgi = pers.tile([64, 4 * CH], BF16, tag="x1q", name="sgi",
                            bufs=4)
            nc.sync.dma_start(
                sgi[:].rearrange("p (i c) -> p i c", i=4),
                h1[b][:, 0:64, cols].rearrange("i p c -> p i c"),
            )
            sgs = pers.tile([64, 4 * CH], BF16, tag="x1q", name="sgs",
                            bufs=4)
            nc.sync.dma_start(
                sgs[:].rearrange("p (i c) -> p i c", i=4),
                h2[:, b * 64:(b + 1) * 64, cols].rearrange("i p c -> p i c"),
            )
            nmm = 0
            for m in range(5):
                # inputs-half: lhsT at rows 0:64, rhs at base 0
                if m == 0:
                    rhs_i = XT[b][0:64, cols]
                else:
                    rhs_i = sgi[:, (m - 1) * CH:m * CH]
                nc.tensor.matmul(
                    pc, WC[0:64, m * 64:(m + 1) * 64], rhs_i,
                    start=(nmm == 0), stop=False,
                )
                nmm += 1
                # states-half: stage everything at base 0 so every matmul
                # keeps tile_position (0,0)
                if m == 0:
                    sgr = stage.tile([64, CH], BF16, tag="sgr", name="sgr",
                                     bufs=3)
                    nc.vector.tensor_copy(
                        sgr[:], RST[b * 64:(b + 1) * 64, cols]
                    )
                    rhs_s = sgr[:]
                else:
                    rhs_s = sgs[:, (m - 1) * CH:m * CH]
                lhs_s = WC[0:64, 320 + m * 64:320 + (m + 1) * 64]
                nmm += 1
                nc.tensor.matmul(
                    pc, lhs_s, rhs_s, start=False, stop=(nmm == 10),
                )
            ctf = stage.tile([128, CH], F32, tag="f1", name="f1", bufs=2)
            nc.scalar.activation(ctf[64:128, :], pc, AF.Tanh, bias=BC[:])
            t1 = stage.tile([128, CH], F32, tag="f2", name="f2", bufs=2)
            nc.vector.tensor_sub(t1[64:128, :], XT[b][64:128, cols],
                                 ctf[64:128, :])
            t2 = stage.tile([128, CH], F32, tag="f3", name="f3", bufs=2)
            nc.vector.tensor_mul(t2[64:128, :], t1[64:128, :],
                                 RUT[b][64:128, cols])
            otf = stage.tile([128, CH], F32, tag="f4", name="f4", bufs=3)
            nc.vector.tensor_add(otf[64:128, :], ctf[64:128, :],
                                 t2[64:128, :])

            def deferred():
                ont = onat.tile([128, 4 * 64], BF16, tag="on", name="on")
                for tp in range(4):
                    pso = pt.tile([128, 128], F32, tag="tp", name="tp")
                    nc.tensor.transpose(
                        pso[0:128, 0:64],
                        otf[64:128, tp * 128:(tp + 1) * 128],
                        IDF[64:128, 64:128],
                    )
                    nc.vector.tensor_copy(
                        ont[:, tp * 64:(tp + 1) * 64], pso[0:128, 0:64]
                    )
                nc.sync.dma_start(
                    out_ap[b, cc * CH:(cc + 1) * CH, :].rearrange(
                        "(a p) d -> p a d", p=128),
                    ont[:].rearrange("p (a d) -> p a d", a=4),
                )
            return deferred

        pend = []
        for b in range(B2):
            for cc in range(NCH):
                pend.append(d2_chunk(b, cc))
                if len(pend) > 3:
                    pend.pop(0)()
        for fn in pend:
            fn()

    return nc


def _get_nc():
    if "nc" not in _CACHE:
        nc = _build()
        nc.compile()
        _CACHE["nc"] = nc
    return _CACHE["nc"]


def kernel(inputs, states, supports, W_ru, b_ru, W_c, b_c, _trace=False):
    bf = ml_dtypes.bfloat16
    f8 = ml_dtypes.float8_e4m3
    B = inputs.shape[0]
    ncore = 8
    bper = B // ncore

    x_cat32 = np.concatenate([inputs, states], axis=-1)              # [16,N,128]
    x_cat = x_cat32.astype(bf)
    x_cat8 = x_cat32.astype(f8)
    # blocked S^T fp8 [s, g, k, p, a2, c]: supT[s, m, n] with
    # m = a2*256 + k*128 + p, n = g*GC + c
    supT8 = (np.asarray(supports).transpose(0, 2, 1) * S_SC).astype(f8)
    supB = np.ascontiguousarray(
        supT8.reshape(NSUP, ND, 2, 128, NG, GC).transpose(0, 4, 2, 3, 1, 5))
    wru = np.asarray(W_ru).astype(bf)
    wc = np.asarray(W_c).astype(bf)
    bru = np.asarray(b_ru).astype(np.float32).reshape(2 * H, 1)
    bc = np.asarray(b_c).astype(np.float32).reshape(H, 1)

    nc = _get_nc()
    in_maps = []
    for c in range(ncore):
        in_maps.append({
            "xcat": np.ascontiguousarray(x_cat[c * bper:(c + 1) * bper]),
            "xcat8": np.ascontiguousarray(x_cat8[c * bper:(c + 1) * bper]),
            "supB": supB,
            "wru": wru,
            "wc": wc,
            "bru": bru,
            "bc": bc,
        })
    res = run_bass_kernel_spmd(
        nc, in_maps, core_ids=list(range(ncore)), trace=_trace,
    )
    outs = [r["out"] for r in res.results]
    full = np.concatenate(outs, axis=0).astype(np.float32)           # [16,N,64]
    if _trace:
        kernel.last_results = res
    return full, full


# revision 37
# speedup vs baseline: 1.0582x; 1.0582x over previous
"""DCGRU cell Trainium2 kernel: batch-parallel SPMD over 8 NeuronCores.

Sharding: data-parallel over batch B=16 -> 2 batches/core; supports and
weights replicated. No collectives.

The 4 diffusion product streams (A1/A2 for gconv1, B1/B2 for gconv2)
dominate: each streams S^T once. They run in fp8e4m3 with DoubleRow
perf mode (contraction of two 128-node blocks per matmul), halving both
HBM traffic (64MB/support/stream) and PE time vs bf16. S is scaled by
2^12 before fp8 quantization (its values ~2^-12 would flush to zero);
hop-1 outputs are re-quantized to fp8 with a 2^6 scale. Dense phases
(D1 ru/sigmoid, D2 c/tanh/blend) stay bf16.

Orientation: stationary lhsT = x[m-dblock, d] fp8 natural layout,
moving rhs = S^T[m-dblock, n_cols] fp8, psum = (S@x)^T [d, n] f32.

Per-core phases:
  0:  XT = x^T via DMA-transpose; X8 = fp8 x natural (host-cast)
  A1: x1_s^T = (S_s@x)^T        -> h1 DRAM bf16 + X1q natural fp8
  A2: x2_s^T = 2(S_s@x1)^T - x^T -> h1
  D1: ru^T = sigmoid(W_ru^T h^T + b); rs^T; XC2 = rs natural fp8 packed
  B1: x1'_s^T = (S_s@rs)^T packed -> h2 + XC3 natural fp8
  B2: x2'_s^T = 2(S_s@x1')^T - rs^T -> h2
  D2: c^T = tanh(W_c^T h'^T + b_c), out^T = c + u*(s - c), PE-transpose,
      DMA out. (inputs-half feats of gconv2 reuse gconv1's h1 rows 0:64)
"""

import sys

sys.path.insert(0, "/opt/trn_rl_repo")

from contextlib import ExitStack

import ml_dtypes
import numpy as np

import concourse.bacc as bacc
import concourse.bass as bass
import concourse.mybir as mybir
import concourse.tile as tile
from concourse.bass_utils import run_bass_kernel_spmd

BF16 = mybir.dt.bfloat16
F32 = mybir.dt.float32
F8 = mybir.dt.float8e4
AF = mybir.ActivationFunctionType
ALU = mybir.AluOpType
DR = mybir.MatmulPerfMode.DoubleRow

N = 8192
DC = 128          # D_IN + D_H
H = 64
B2 = 2            # batches per core
NBLK = N // 128   # 64 m-blocks
ND = NBLK // 2    # 32 double m-blocks (DoubleRow)
CH = 512          # psum chunk (free dim)
NCH = N // CH     # 16 chunks
# groups of chunks sharing one stationary load; 4 product psum banks
GCN = 2           # chunks per group
GC = GCN * CH     # group columns (1024)
NG = NCH // GCN   # 8 groups
OCT = 4           # double m-blocks per stream DMA (4KB contiguous runs)
NSUP = 2

S_SC = 2.0 ** 12   # host scale on S before fp8 quantization
X_SC = 2.0 ** 6    # scale on hop-1 outputs for fp8 re-quantization

_CACHE = {}


def _build():
    import os
    PHASES = int(os.environ.get("DCGRU_PHASES", "6"))
    nc = bacc.Bacc("TRN2", target_bir_lowering=False, debug=False)

    xc_d = nc.dram_tensor("xcat", [B2, N, DC], BF16, kind="ExternalInput")
    xc8_d = nc.dram_tensor("xcat8", [B2, N, DC], F8, kind="ExternalInput")
    # blocked S^T fp8: [s, group, k, p, a2, c]; per (g,k,p) the (a2,c)
    # range is contiguous, giving 4KB DMA descriptor runs
    sup_d = nc.dram_tensor("supB", [NSUP, NG, 2, 128, ND, GC], F8,
                           kind="ExternalInput")
    wru_d = nc.dram_tensor("wru", [5 * DC, 2 * H], BF16, kind="ExternalInput")
    wc_d = nc.dram_tensor("wc", [5 * DC, H], BF16, kind="ExternalInput")
    bru_d = nc.dram_tensor("bru", [2 * H, 1], F32, kind="ExternalInput")
    bc_d = nc.dram_tensor("bc", [H, 1], F32, kind="ExternalInput")
    out_d = nc.dram_tensor("out", [B2, N, H], BF16, kind="ExternalOutput")

    id_bf = nc.inline_tensor(np.eye(128, dtype=ml_dtypes.bfloat16), "id_bf")
    id_f = nc.inline_tensor(np.eye(128, dtype=np.float32), "id_f")
    id_8 = nc.inline_tensor(np.eye(128, dtype=ml_dtypes.float8_e4m3), "id_8")

    xc_ap = xc_d.ap()
    xc8_ap = xc8_d.ap()
    sup_ap = sup_d.ap()
    out_ap = out_d.ap()

    with tile.TileContext(nc) as tc, ExitStack() as ctx:
        cpool = ctx.enter_context(tc.tile_pool(name="const", bufs=1))
        dram = ctx.enter_context(tc.tile_pool(name="dram", bufs=1, space="DRAM"))
        pers = ctx.enter_context(tc.tile_pool(name="pers", bufs=1))
        st = ctx.enter_context(tc.tile_pool(name="st", bufs=3))
        stage = ctx.enter_context(tc.tile_pool(name="stage", bufs=10))
        onat = ctx.enter_context(tc.tile_pool(name="onat", bufs=4))
        pp = ctx.enter_context(tc.tile_pool(name="pp", bufs=4, space="PSUM"))
        pt = ctx.enter_context(tc.tile_pool(name="pt", bufs=4, space="PSUM"))

        # ---- constants: only ID8 loads up front (needed by A1's deferred
        # transposes); the rest queue after A1's stream emission so the
        # first stream DMAs aren't delayed behind them ----
        IDB = cpool.tile([128, 128], BF16, tag="idb", name="idb")
        IDF = cpool.tile([128, 128], F32, tag="idf", name="idf")
        ID8 = cpool.tile([128, 128], F8, tag="id8", name="id8")
        nc.sync.dma_start(ID8[:], id_8.ap())
        WRU = cpool.tile([128, 5 * 128], BF16, tag="wru", name="wru")
        # WC layout: cols m*64:(m+1)*64 = inputs-half block (rows 0:64);
        # cols 320+m*64 = states-half block, duplicated at rows 0:64 and 64:128
        WC = cpool.tile([128, 10 * 64], BF16, tag="wc", name="wc")
        BRU = cpool.tile([128, 1], F32, tag="bru", name="bru")
        BC = cpool.tile([64, 1], F32, tag="bc", name="bc")

        def load_late_consts():
            nc.sync.dma_start(IDB[:], id_bf.ap())
            nc.sync.dma_start(IDF[:], id_f.ap())
            nc.sync.dma_start(
                WRU[:].rearrange("p (a o) -> p a o", a=5),
                wru_d.ap().rearrange("(a p) o -> p a o", p=128),
            )
            for m in range(5):
                nc.sync.dma_start(
                    WC[0:64, m * 64:(m + 1) * 64],
                    wc_d.ap()[m * 128:m * 128 + 64, :]
                )
                nc.sync.dma_start(
                    WC[0:64, 320 + m * 64:320 + (m + 1) * 64],
                    wc_d.ap()[m * 128 + 64:(m + 1) * 128, :],
                )
                nc.sync.dma_start(
                    WC[64:128, 320 + m * 64:320 + (m + 1) * 64],
                    wc_d.ap()[m * 128 + 64:(m + 1) * 128, :],
                )
            nc.sync.dma_start(BRU[:], bru_d.ap())
            nc.sync.dma_start(BC[:], bc_d.ap())
            for b in range(B2):
                nc.sync.dma_start_transpose(XT[b][:], xc_ap[b])

        # ---- DRAM scratch: gconv1 product feats^T, i = 2s+hop:
        # (x1_s0, x2_s0, x1_s1, x2_s1) stacked for batched D-phase loads
        h1 = [dram.tile([4, 128, N], BF16, tag=f"h1_{b}", name=f"h1_{b}")
              for b in range(B2)]
        # gconv2 states-half feats^T, batch-packed rows (b*64), i = 2s+hop
        h2 = dram.tile([4, 128, N], BF16, tag="h2", name="h2")

        # ---- persistent SBUF ----
        XT = [pers.tile([128, N], BF16, tag="xt", name=f"XT_{b}", bufs=2)
              for b in range(B2)]
        X8 = [pers.tile([128, N], F8, tag="x8", name=f"X8_{b}", bufs=2)
              for b in range(B2)]
        X1q = [[pers.tile([128, N], F8, tag="x1q", name=f"X1q_{s}_{b}", bufs=4)
                for b in range(B2)] for s in range(NSUP)]

        # ---- phase 0: x natural fp8 from host (x^T DMA-transposes queue
        # in load_late_consts, after A1) ----
        for b in range(B2):
            nc.sync.dma_start(
                X8[b][:].rearrange("p (a d) -> p a d", a=NBLK),
                xc8_ap[b].rearrange("(a p) d -> p a d", p=128),
            )

        def dr_slice(T, a2):
            """[128, 2, 128] DoubleRow lhsT view of natural-layout tile T."""
            return T[:, a2 * 256:(a2 + 1) * 256].rearrange(
                "p (k d) -> p k d", k=2)

        def product_stream(lhs_of, psum_sink, pack_batches):
            """Stream supT once (fp8, DoubleRow over double m-blocks).

            lhs_of(s, b, a2) -> lhsT AP [128, 2, 128]. psum_sink(s,
            b_or_None, j, c0, cnt, psum) consumes the finished [128, CH]
            f32 psum for chunk c0+j and returns a deferred closure (PE
            transpose tail) or None. Deferred work is emitted after the
            NEXT group's matmuls so the PE never waits on the ACT/DVE
            psum-drain chain.
            """
            # supports interleaved per group so both supports' outputs for a
            # given column range complete early -> downstream dense phases
            # (D1/D2) overlap the stream instead of waiting for its end
            pending = []
            for g in range(NG):
                for s in range(NSUP):
                    if pack_batches:
                        psums = [pp.tile([128, CH], F32, tag="pp", name="pp")
                                 for j in range(GCN)]
                    else:
                        psums = [pp.tile([128, CH], F32, tag="pp", name="pp")
                                 for _ in range(B2 * GCN)]
                    for o in range(ND // OCT):
                        stt = st.tile([128, 2 * OCT * GC], F8, tag="st",
                                      name="st")
                        st4 = stt[:].rearrange("p (k a c) -> p k a c",
                                               k=2, a=OCT)
                        nc.sync.dma_start(
                            st4,
                            sup_ap[s, g, :, :, o * OCT:(o + 1) * OCT,
                                   :].rearrange("k p a c -> p k a c"),
                        )
                        for a in range(OCT):
                            a2 = o * OCT + a
                            first = a2 == 0
                            last = a2 == ND - 1
                            if pack_batches:
                                lhsT = lhs_of(s, None, a2)
                                for j in range(GCN):
                                    nc.tensor.matmul(
                                        psums[j][:], lhsT,
                                        st4[:, :, a, j * CH:(j + 1) * CH],
                                        start=first, stop=last, perf_mode=DR,
                                    )
                            else:
                                for b in range(B2):
                                    lhsT = lhs_of(s, b, a2)
                                    for j in range(GCN):
                                        nc.tensor.matmul(
                                            psums[b * GCN + j][:], lhsT,
                                            st4[:, :, a, j * CH:(j + 1) * CH],
                                            start=first, stop=last,
                                            perf_mode=DR,
                                        )
                    # previous group's transpose tails land after this
                    # group's matmuls in the PE queue
                    for fn in pending:
                        fn()
                    pending = []
                    if pack_batches:
                        for j in range(GCN):
                            d = psum_sink(s, None, g * GCN + j, psums[j])
                            if d:
                                pending.append(d)
                    else:
                        for b in range(B2):
                            for j in range(GCN):
                                d = psum_sink(s, b, g * GCN + j,
                                              psums[b * GCN + j])
                                if d:
                                    pending.append(d)
            for fn in pending:
                fn()

        def hop1_sink(h_dst, q_dst):
            """psum = 2^12 (S@x): h_dst gets unscaled bf16 ^T feats, q_dst
            gets fp8 2^6-scaled natural layout via PE transposes."""
            def sink(s, b, cc, psum):
                cols = slice(cc * CH, (cc + 1) * CH)
                t = stage.tile([128, CH], BF16, tag="sg", name="sg")
                nc.scalar.activation(t[:], psum[:], AF.Copy, scale=1.0 / S_SC)
                nc.sync.dma_start(h_dst(s, b)[:, cols], t[:])
                t8 = stage.tile([128, CH], F8, tag="s8", name="s8", bufs=10)
                nc.vector.tensor_scalar_mul(t8[:], psum[:], X_SC / S_SC)

                def deferred():
                    for tp in range(4):
                        blk = cc * 4 + tp
                        ps8 = pt.tile([128, 256], F8, tag="tp", name="tp")
                        ps8_s = ps8[:].rearrange("p (c two) -> p c two", two=2)[:, :, 0]
                        nc.tensor.transpose(
                            ps8_s, t8[:, tp * 128:(tp + 1) * 128], ID8[:]
                        )
                        nc.vector.tensor_copy(
                            q_dst(s, b)[:, blk * 128:(blk + 1) * 128], ps8_s
                        )
                return deferred
            return sink

        def hop2_sink(h_dst, sub_of):
            """psum = 2^18 (S@x1): x2 = psum*2^-17 - sub (bf16 ^T)."""
            def sink(s, b, cc, psum):
                cols = slice(cc * CH, (cc + 1) * CH)
                t = stage.tile([128, CH], BF16, tag="sg", name="sg")
                nc.vector.scalar_tensor_tensor(
                    t[:], psum[:], 2.0 / (S_SC * X_SC), sub_of(b)[:, cols],
                    op0=ALU.mult, op1=ALU.subtract,
                )
                nc.sync.dma_start(h_dst(s, b)[:, cols], t[:])
                return None
            return sink

        # ---- A1: x1_s^T = (S_s @ x)^T ----
        product_stream(
            lambda s, b, a2: dr_slice(X8[b], a2),
            hop1_sink(lambda s, b: h1[b][2 * s], lambda s, b: X1q[s][b]),
            pack_batches=False,
        )
        load_late_consts()

        if PHASES < 2:
            return nc
        # ---- A2: x2_s^T = 2*(S_s @ x1_s)^T - x^T ----
        product_stream(
            lambda s, b, a2: dr_slice(X1q[s][b], a2),
            hop2_sink(lambda s, b: h1[b][2 * s + 1], lambda b: XT[b]),
            pack_batches=False,
        )

        if PHASES < 3:
            return nc
        # ---- D1: dense ru + sigmoid + rs^T + XC2 natural fp8 ----
        RUT = [pers.tile([128, N], BF16, tag="big2", name=f"RUT_{b}", bufs=2)
               for b in range(B2)]
        RST = pers.tile([128, N], BF16, tag="rst", name="RST")
        # XC2/XC3 reuse X8's two fp8 slots (X8 is dead after A1)
        XC2 = pers.tile([128, N], F8, tag="x8", name="XC2", bufs=2)
        def d1_chunk(b, cc):
            cols = slice(cc * CH, (cc + 1) * CH)
            ps = pp.tile([128, CH], F32, tag="pp", name="pp")
            # one batched load of all 4 product feats (reuses x1q slots)
            sg4 = pers.tile([128, 4 * CH], BF16, tag="x1q", name="sg4",
                            bufs=4)
            nc.sync.dma_start(
                sg4[:].rearrange("p (i c) -> p i c", i=4),
                h1[b][:, :, cols].rearrange("i p c -> p i c"),
            )
            for i in range(5):
                if i == 0:
                    rhs = XT[b][:, cols]
                else:
                    rhs = sg4[:, (i - 1) * CH:i * CH]
                nc.tensor.matmul(
                    ps[:], WRU[:, i * 128:(i + 1) * 128], rhs,
                    start=(i == 0), stop=(i == 4),
                )
            nc.scalar.activation(
                RUT[b][:, cols], ps[:], AF.Sigmoid, bias=BRU[:]
            )
            # rs = r * states^T; base-shift states^T and the result via
            # single-input copies (SB-SB two-input ops need equal bases)
            sts = stage.tile([64, CH], BF16, tag="sh1", name="sh1", bufs=3)
            nc.vector.tensor_copy(sts[:], XT[b][64:128, cols])
            rsc = stage.tile([64, CH], BF16, tag="sh2", name="sh2", bufs=3)
            nc.vector.tensor_mul(rsc[:], RUT[b][0:64, cols], sts[:])
            nc.vector.tensor_copy(RST[b * 64:(b + 1) * 64, cols], rsc[:])

            def deferred():
                for tp in range(4):
                    blk = cc * 4 + tp
                    ps2 = pt.tile([128, 128], BF16, tag="tp", name="tp")
                    nc.tensor.transpose(
                        ps2[0:128, 0:64],
                        RST[b * 64:(b + 1) * 64, blk * 128:(blk + 1) * 128],
                        IDB[b * 64:(b + 1) * 64, b * 64:b * 64 + 64],
                    )
                    nc.vector.tensor_copy(
                        XC2[:, blk * 128 + b * 64:blk * 128 + b * 64 + 64],
                        ps2[0:128, 0:64],
                    )
            return deferred

        pend = []
        for b in range(B2):
            for cc in range(NCH):
                pend.append(d1_chunk(b, cc))
                if len(pend) > 3:
                    pend.pop(0)()
        for fn in pend:
            fn()

        if PHASES < 4:
            return nc
        # ---- B1: x1'_s^T packed = (S_s @ rs)^T ----
        XC3 = pers.tile([128, N], F8, tag="x8", name="XC3", bufs=2)
        product_stream(
            lambda s, b, a2: dr_slice(XC2, a2),
            hop1_sink(lambda s, b: h2[2 * s], lambda s, b: XC3),
            pack_batches=True,
        )

        if PHASES < 5:
            return nc
        # ---- B2: x2'_s^T packed = 2*(S_s @ x1')^T - rs^T ----
        product_stream(
            lambda s, b, a2: dr_slice(XC3, a2),
            hop2_sink(lambda s, b: h2[2 * s + 1], lambda b: RST),
            pack_batches=True,
        )

        if PHASES < 6:
            return nc
        # ---- D2: dense c + tanh + blend + transpose + out ----
        # blend runs at base-64 partitions so the two-input DVE ops read
        # XT/RUT rows 64:128 directly (no alignment copies)
        def d2_chunk(b, cc):
            cols = slice(cc * CH, (cc + 1) * CH)
            ps = pp.tile([128, CH], F32, tag="pp", name="pp")
            pc = ps[0:64, :]
            # batched loads: 4 inputs-half feats (h1 rows 0:64) and 4
            # states-half feats (h2 rows b*64:), reusing x1q slots
            sgi = pers.tile([64, 4 * CH], BF16, tag="x1q", name="sgi",
                            bufs=4)
            nc.sync.dma_start(
                sgi[:].rearrange("p (i c) -> p i c", i=4),
                h1[b][:, 0:64, cols].rearrange("i p c -> p i c"),
            )
            sgs = pers.tile([64, 4 * CH], BF16, tag="x1q", name="sgs",
                            bufs=4)
            nc.sync.dma_start(
                sgs[:].rearrange("p (i c) -> p i c", i=4),
                h2[:, b * 64:(b + 1) * 64, cols].rearrange("i p c -> p i c"),
            )
            nmm = 0
            for m in range(5):
                # inputs-half: lhsT at rows 0:64, rhs at base 0
                if m == 0:
                    rhs_i = XT[b][0:64, cols]
                else:
                    rhs_i = sgi[:, (m - 1) * CH:m * CH]
                nc.tensor.matmul(
                    pc, WC[0:64, m * 64:(m + 1) * 64], rhs_i,
                    start=(nmm == 0), stop=False,
                )
                nmm += 1
                # states-half: stage everything at base 0 so every matmul
                # keeps tile_position (0,0)
                if m == 0:
                    sgr = stage.tile([64, CH], BF16, tag="sgr", name="sgr",
                                     bufs=3)
                    nc.vector.tensor_copy(
                        sgr[:], RST[b * 64:(b + 1) * 64, cols]
                    )
                    rhs_s = sgr[:]
                else:
                    rhs_s = sgs[:, (m - 1) * CH:m * CH]
                lhs_s = WC[0:64, 320 + m * 64:320 + (m + 1) * 64]
                nmm += 1
                nc.tensor.matmul(
                    pc, lhs_s, rhs_s, start=False, stop=(nmm == 10),
                )
            ctf = stage.tile([128, CH], F32, tag="f1", name="f1", bufs=2)
            nc.scalar.activation(ctf[64:128, :], pc, AF.Tanh, bias=BC[:])
            t1 = stage.tile([128, CH], F32, tag="f2", name="f2", bufs=2)
            nc.vector.tensor_sub(t1[64:128, :], XT[b][64:128, cols],
                                 ctf[64:128, :])
            t2 = stage.tile([128, CH], F32, tag="f3", name="f3", bufs=2)
            nc.vector.tensor_mul(t2[64:128, :], t1[64:128, :],
                                 RUT[b][64:128, cols])
            otf = stage.tile([128, CH], F32, tag="f4", name="f4", bufs=3)
            nc.vector.tensor_add(otf[64:128, :], ctf[64:128, :],
                                 t2[64:128, :])

            def deferred():
                ont = onat.tile([128, 4 * 64], BF16, tag="on", name="on")
                for tp in range(4):
                    pso = pt.tile([128, 128], F32, tag="tp", name="tp")
                    nc.tensor.transpose(
                        pso[0:128, 0:64],
                        otf[64:128, tp * 128:(tp + 1) * 128],
                        IDF[64:128, 64:128],
                    )
                    nc.vector.tensor_copy(
                        ont[:, tp * 64:(tp + 1) * 64], pso[0:128, 0:64]
                    )
                nc.sync.dma_start(
                    out_ap[b, cc * CH:(cc + 1) * CH, :].rearrange(
                        "(a p) d -> p a d", p=128),
                    ont[:].rearrange("p (a d) -> p a d", a=4),
                )
            return deferred

        pend = []
        for b in range(B2):
            for cc in range(NCH):
                pend.append(d2_chunk(b, cc))
                if len(pend) > 3:
                    pend.pop(0)()
        for fn in pend:
            fn()

    return nc


def _get_nc():
    if "nc" not in _CACHE:
        nc = _build()
        nc.compile()
        _CACHE["nc"] = nc
    return _CACHE["nc"]


def kernel(inputs, states, supports, W_ru, b_ru, W_c, b_c, _trace=False):
    bf = ml_dtypes.bfloat16
    f8 = ml_dtypes.float8_e4m3
    B = inputs.shape[0]
    ncore = 8
    bper = B // ncore

    x_cat32 = np.concatenate([inputs, states], axis=-1)              # [16,N,128]
    x_cat = x_cat32.astype(bf)
    x_cat8 = x_cat32.astype(f8)
    # blocked S^T fp8 [s, g, k, p, a2, c]: supT[s, m, n] with
    # m = a2*256 + k*128 + p, n = g*GC + c
    supT8 = (np.asarray(supports).transpose(0, 2, 1) * S_SC).astype(f8)
    supB = np.ascontiguousarray(
        supT8.reshape(NSUP, ND, 2, 128, NG, GC).transpose(0, 4, 2, 3, 1, 5))
    wru = np.asarray(W_ru).astype(bf)
    wc = np.asarray(W_c).astype(bf)
    bru = np.asarray(b_ru).astype(np.float32).reshape(2 * H, 1)
    bc = np.asarray(b_c).astype(np.float32).reshape(H, 1)

    nc = _get_nc()
    in_maps = []
    for c in range(ncore):
        in_maps.append({
            "xcat": np.ascontiguousarray(x_cat[c * bper:(c + 1) * bper]),
            "xcat8": np.ascontiguousarray(x_cat8[c * bper:(c + 1) * bper]),
            "supB": supB,
            "wru": wru,
            "wc": wc,
            "bru": bru,
            "bc": bc,
        })
    res = run_bass_kernel_spmd(
        nc, in_maps, core_ids=list(range(ncore)), trace=_trace,
    )
    outs = [r["out"] for r in res.results]
    full = np.concatenate(outs, axis=0).astype(np.float32)           # [16,N,64]
    if _trace:
        kernel.last_results = res
    return full, full


# revision 38
# speedup vs baseline: 1.1360x; 1.0735x over previous
"""DCGRU cell Trainium2 kernel: batch-parallel SPMD over 8 NeuronCores.

Sharding: data-parallel over batch B=16 -> 2 batches/core; supports and
weights replicated. No collectives.

The 4 diffusion product streams (A1/A2 for gconv1, B1/B2 for gconv2)
dominate: each streams S^T once. They run in fp8e4m3 with DoubleRow
perf mode (contraction of two 128-node blocks per matmul), halving both
HBM traffic (64MB/support/stream) and PE time vs bf16. S is scaled by
2^12 before fp8 quantization (its values ~2^-12 would flush to zero);
hop-1 outputs are re-quantized to fp8 with a 2^6 scale. Dense phases
(D1 ru/sigmoid, D2 c/tanh/blend) stay bf16.

Orientation: stationary lhsT = x[m-dblock, d] fp8 natural layout,
moving rhs = S^T[m-dblock, n_cols] fp8, psum = (S@x)^T [d, n] f32.

Per-core phases:
  0:  XT = x^T via DMA-transpose; X8 = fp8 x natural (host-cast)
  A1: x1_s^T = (S_s@x)^T        -> h1 DRAM bf16 + X1q natural fp8
  A2: x2_s^T = 2(S_s@x1)^T - x^T -> h1
  D1: ru^T = sigmoid(W_ru^T h^T + b); rs^T; XC2 = rs natural fp8 packed
  B1: x1'_s^T = (S_s@rs)^T packed -> h2 + XC3 natural fp8
  B2: x2'_s^T = 2(S_s@x1')^T - rs^T -> h2
  D2: c^T = tanh(W_c^T h'^T + b_c), out^T = c + u*(s - c), PE-transpose,
      DMA out. (inputs-half feats of gconv2 reuse gconv1's h1 rows 0:64)
"""

import sys

sys.path.insert(0, "/opt/trn_rl_repo")

from contextlib import ExitStack

import ml_dtypes
import numpy as np

import concourse.bacc as bacc
import concourse.bass as bass
import concourse.mybir as mybir
import concourse.tile as tile
from concourse.bass_utils import run_bass_kernel_spmd

BF16 = mybir.dt.bfloat16
F32 = mybir.dt.float32
F8 = mybir.dt.float8e4
AF = mybir.ActivationFunctionType
ALU = mybir.AluOpType
DR = mybir.MatmulPerfMode.DoubleRow

N = 8192
DC = 128          # D_IN + D_H
H = 64
B2 = 2            # batches per core
NBLK = N // 128   # 64 m-blocks
ND = NBLK // 2    # 32 double m-blocks (DoubleRow)
CH = 512          # psum chunk (free dim)
NCH = N // CH     # 16 chunks
# groups of chunks sharing one stationary load; 4 product psum banks
GCN = 2           # chunks per group
GC = GCN * CH     # group columns (1024)
NG = NCH // GCN   # 8 groups
OCT = 4           # double m-blocks per stream DMA (4KB contiguous runs)
NSUP = 2

S_SC = 2.0 ** 12   # host scale on S before fp8 quantization
X_SC = 2.0 ** 6    # scale on hop-1 outputs for fp8 re-quantization

_CACHE = {}


def _build():
    import os
    PHASES = int(os.environ.get("DCGRU_PHASES", "6"))
    nc = bacc.Bacc("TRN2", target_bir_lowering=False, debug=False)

    xc_d = nc.dram_tensor("xcat", [B2, N, DC], BF16, kind="ExternalInput")
    xc8_d = nc.dram_tensor("xcat8", [B2, N, DC], F8, kind="ExternalInput")
    # blocked S^T fp8: [s, group, k, p, a2, c]; per (g,k,p) the (a2,c)
    # range is contiguous, giving 4KB DMA descriptor runs
    sup_d = nc.dram_tensor("supB", [NSUP, NG, 2, 128, ND, GC], F8,
                           kind="ExternalInput")
    wru_d = nc.dram_tensor("wru", [5 * DC, 2 * H], BF16, kind="ExternalInput")
    wc_d = nc.dram_tensor("wc", [5 * DC, H], BF16, kind="ExternalInput")
    bru_d = nc.dram_tensor("bru", [2 * H, 1], F32, kind="ExternalInput")
    bc_d = nc.dram_tensor("bc", [H, 1], F32, kind="ExternalInput")
    out_d = nc.dram_tensor("out", [B2, N, H], BF16, kind="ExternalOutput")

    id_bf = nc.inline_tensor(np.eye(128, dtype=ml_dtypes.bfloat16), "id_bf")
    id_f = nc.inline_tensor(np.eye(128, dtype=np.float32), "id_f")
    id_8 = nc.inline_tensor(np.eye(128, dtype=ml_dtypes.float8_e4m3), "id_8")

    xc_ap = xc_d.ap()
    xc8_ap = xc8_d.ap()
    sup_ap = sup_d.ap()
    out_ap = out_d.ap()

    with tile.TileContext(nc) as tc, ExitStack() as ctx:
        cpool = ctx.enter_context(tc.tile_pool(name="const", bufs=1))
        dram = ctx.enter_context(tc.tile_pool(name="dram", bufs=1, space="DRAM"))
        pers = ctx.enter_context(tc.tile_pool(name="pers", bufs=1))
        st = ctx.enter_context(tc.tile_pool(name="st", bufs=3))
        stage = ctx.enter_context(tc.tile_pool(name="stage", bufs=10))
        onat = ctx.enter_context(tc.tile_pool(name="onat", bufs=4))
        pp = ctx.enter_context(tc.tile_pool(name="pp", bufs=4, space="PSUM"))
        pt = ctx.enter_context(tc.tile_pool(name="pt", bufs=4, space="PSUM"))

        # ---- constants ----
        IDB = cpool.tile([128, 128], BF16, tag="idb", name="idb")
        nc.sync.dma_start(IDB[:], id_bf.ap())
        IDF = cpool.tile([128, 128], F32, tag="idf", name="idf")
        nc.sync.dma_start(IDF[:], id_f.ap())
        ID8 = cpool.tile([128, 128], F8, tag="id8", name="id8")
        nc.sync.dma_start(ID8[:], id_8.ap())
        WRU = cpool.tile([128, 5 * 128], BF16, tag="wru", name="wru")
        nc.sync.dma_start(
            WRU[:].rearrange("p (a o) -> p a o", a=5),
            wru_d.ap().rearrange("(a p) o -> p a o", p=128),
        )
        # WC layout: cols m*64:(m+1)*64 = inputs-half block (rows 0:64);
        # cols 320+m*64 = states-half block, duplicated at rows 0:64 and 64:128
        WC = cpool.tile([128, 10 * 64], BF16, tag="wc", name="wc")
        for m in range(5):
            nc.sync.dma_start(
                WC[0:64, m * 64:(m + 1) * 64], wc_d.ap()[m * 128:m * 128 + 64, :]
            )
            nc.sync.dma_start(
                WC[0:64, 320 + m * 64:320 + (m + 1) * 64],
                wc_d.ap()[m * 128 + 64:(m + 1) * 128, :],
            )
            nc.sync.dma_start(
                WC[64:128, 320 + m * 64:320 + (m + 1) * 64],
                wc_d.ap()[m * 128 + 64:(m + 1) * 128, :],
            )
        BRU = cpool.tile([128, 1], F32, tag="bru", name="bru")
        nc.sync.dma_start(BRU[:], bru_d.ap())
        BC = cpool.tile([64, 1], F32, tag="bc", name="bc")
        nc.sync.dma_start(BC[:], bc_d.ap())

        # ---- DRAM scratch: gconv1 product feats^T, i = 2s+hop:
        # (x1_s0, x2_s0, x1_s1, x2_s1) stacked for batched D-phase loads
        h1 = [dram.tile([4, 128, N], BF16, tag=f"h1_{b}", name=f"h1_{b}")
              for b in range(B2)]
        # gconv2 states-half feats^T, batch-packed rows (b*64), i = 2s+hop
        h2 = dram.tile([4, 128, N], BF16, tag="h2", name="h2")

        # ---- persistent SBUF ----
        XT = [pers.tile([128, N], BF16, tag="xt", name=f"XT_{b}", bufs=2)
              for b in range(B2)]
        X8 = [pers.tile([128, N], F8, tag="x8", name=f"X8_{b}", bufs=2)
              for b in range(B2)]
        X1q = [[pers.tile([128, N], F8, tag="x1q", name=f"X1q_{s}_{b}", bufs=4)
                for b in range(B2)] for s in range(NSUP)]

        # ---- phase 0: x^T via DMA transpose, x natural fp8 from host ----
        for b in range(B2):
            nc.sync.dma_start_transpose(XT[b][:], xc_ap[b])
            nc.sync.dma_start(
                X8[b][:].rearrange("p (a d) -> p a d", a=NBLK),
                xc8_ap[b].rearrange("(a p) d -> p a d", p=128),
            )

        def dr_slice(T, a2):
            """[128, 2, 128] DoubleRow lhsT view of natural-layout tile T."""
            return T[:, a2 * 256:(a2 + 1) * 256].rearrange(
                "p (k d) -> p k d", k=2)

        def product_stream(lhs_of, psum_sink, pack_batches):
            """Stream supT once (fp8, DoubleRow over double m-blocks).

            lhs_of(s, b, a2) -> lhsT AP [128, 2, 128]. psum_sink(s,
            b_or_None, j, c0, cnt, psum) consumes the finished [128, CH]
            f32 psum for chunk c0+j and returns a deferred closure (PE
            transpose tail) or None. Deferred work is emitted after the
            NEXT group's matmuls so the PE never waits on the ACT/DVE
            psum-drain chain.
            """
            # supports interleaved per group so both supports' outputs for a
            # given column range complete early -> downstream dense phases
            # (D1/D2) overlap the stream instead of waiting for its end
            pending = []
            for g in range(NG):
                for s in range(NSUP):
                    if pack_batches:
                        psums = [pp.tile([128, CH], F32, tag="pp", name="pp")
                                 for j in range(GCN)]
                    else:
                        psums = [pp.tile([128, CH], F32, tag="pp", name="pp")
                                 for _ in range(B2 * GCN)]
                    for o in range(ND // OCT):
                        stt = st.tile([128, 2 * OCT * GC], F8, tag="st",
                                      name="st")
                        st4 = stt[:].rearrange("p (k a c) -> p k a c",
                                               k=2, a=OCT)
                        nc.sync.dma_start(
                            st4,
                            sup_ap[s, g, :, :, o * OCT:(o + 1) * OCT,
                                   :].rearrange("k p a c -> p k a c"),
                        )
                        for a in range(OCT):
                            a2 = o * OCT + a
                            first = a2 == 0
                            last = a2 == ND - 1
                            if pack_batches:
                                lhsT = lhs_of(s, None, a2)
                                for j in range(GCN):
                                    nc.tensor.matmul(
                                        psums[j][:], lhsT,
                                        st4[:, :, a, j * CH:(j + 1) * CH],
                                        start=first, stop=last, perf_mode=DR,
                                    )
                            else:
                                for b in range(B2):
                                    lhsT = lhs_of(s, b, a2)
                                    for j in range(GCN):
                                        nc.tensor.matmul(
                                            psums[b * GCN + j][:], lhsT,
                                            st4[:, :, a, j * CH:(j + 1) * CH],
                                            start=first, stop=last,
                                            perf_mode=DR,
                                        )
                    # previous group's transpose tails land after this
                    # group's matmuls in the PE queue
                    for fn in pending:
                        fn()
                    pending = []
                    if pack_batches:
                        for j in range(GCN):
                            d = psum_sink(s, None, g * GCN + j, psums[j])
                            if d:
                                pending.append(d)
                    else:
                        for b in range(B2):
                            for j in range(GCN):
                                d = psum_sink(s, b, g * GCN + j,
                                              psums[b * GCN + j])
                                if d:
                                    pending.append(d)
            for fn in pending:
                fn()

        def hop1_sink(h_dst, q_dst):
            """psum = 2^12 (S@x): h_dst gets unscaled bf16 ^T feats, q_dst
            gets fp8 2^6-scaled natural layout via PE transposes."""
            def sink(s, b, cc, psum):
                cols = slice(cc * CH, (cc + 1) * CH)
                t = stage.tile([128, CH], BF16, tag="sg", name="sg")
                nc.scalar.activation(t[:], psum[:], AF.Copy, scale=1.0 / S_SC)
                nc.sync.dma_start(h_dst(s, b)[:, cols], t[:])
                t8 = stage.tile([128, CH], F8, tag="s8", name="s8", bufs=10)
                nc.vector.tensor_scalar_mul(t8[:], psum[:], X_SC / S_SC)

                def deferred():
                    for tp in range(4):
                        blk = cc * 4 + tp
                        ps8 = pt.tile([128, 256], F8, tag="tp", name="tp")
                        ps8_s = ps8[:].rearrange("p (c two) -> p c two", two=2)[:, :, 0]
                        nc.tensor.transpose(
                            ps8_s, t8[:, tp * 128:(tp + 1) * 128], ID8[:]
                        )
                        nc.vector.tensor_copy(
                            q_dst(s, b)[:, blk * 128:(blk + 1) * 128], ps8_s
                        )
                return deferred
            return sink

        def hop2_sink(h_dst, sub_of):
            """psum = 2^18 (S@x1): x2 = psum*2^-17 - sub (bf16 ^T)."""
            def sink(s, b, cc, psum):
                cols = slice(cc * CH, (cc + 1) * CH)
                t = stage.tile([128, CH], BF16, tag="sg", name="sg")
                nc.vector.scalar_tensor_tensor(
                    t[:], psum[:], 2.0 / (S_SC * X_SC), sub_of(b)[:, cols],
                    op0=ALU.mult, op1=ALU.subtract,
                )
                nc.sync.dma_start(h_dst(s, b)[:, cols], t[:])
                return None
            return sink

        # ---- A1: x1_s^T = (S_s @ x)^T ----
        product_stream(
            lambda s, b, a2: dr_slice(X8[b], a2),
            hop1_sink(lambda s, b: h1[b][2 * s], lambda s, b: X1q[s][b]),
            pack_batches=False,
        )

        if PHASES < 2:
            return nc
        # ---- A2: x2_s^T = 2*(S_s @ x1_s)^T - x^T ----
        product_stream(
            lambda s, b, a2: dr_slice(X1q[s][b], a2),
            hop2_sink(lambda s, b: h1[b][2 * s + 1], lambda b: XT[b]),
            pack_batches=False,
        )

        if PHASES < 3:
            return nc
        # ---- D1: dense ru + sigmoid + rs^T + XC2 natural fp8 ----
        RUT = [pers.tile([128, N], BF16, tag="big2", name=f"RUT_{b}", bufs=2)
               for b in range(B2)]
        RST = pers.tile([128, N], BF16, tag="rst", name="RST")
        # XC2/XC3 reuse X8's two fp8 slots (X8 is dead after A1)
        XC2 = pers.tile([128, N], F8, tag="x8", name="XC2", bufs=2)
        def d1_chunk(b, cc):
            cols = slice(cc * CH, (cc + 1) * CH)
            ps = pp.tile([128, CH], F32, tag="pp", name="pp")
            # one batched load of all 4 product feats (reuses x1q slots)
            sg4 = pers.tile([128, 4 * CH], BF16, tag="x1q", name="sg4",
                            bufs=4)
            nc.sync.dma_start(
                sg4[:].rearrange("p (i c) -> p i c", i=4),
                h1[b][:, :, cols].rearrange("i p c -> p i c"),
            )
            for i in range(5):
                if i == 0:
                    rhs = XT[b][:, cols]
                else:
                    rhs = sg4[:, (i - 1) * CH:i * CH]
                nc.tensor.matmul(
                    ps[:], WRU[:, i * 128:(i + 1) * 128], rhs,
                    start=(i == 0), stop=(i == 4),
                )
            nc.scalar.activation(
                RUT[b][:, cols], ps[:], AF.Sigmoid, bias=BRU[:]
            )
            # rs = r * states^T; base-shift states^T and the result via
            # single-input copies (SB-SB two-input ops need equal bases)
            sts = stage.tile([64, CH], BF16, tag="sh1", name="sh1", bufs=3)
            nc.vector.tensor_copy(sts[:], XT[b][64:128, cols])
            rsc = stage.tile([64, CH], BF16, tag="sh2", name="sh2", bufs=3)
            nc.vector.tensor_mul(rsc[:], RUT[b][0:64, cols], sts[:])
            nc.vector.tensor_copy(RST[b * 64:(b + 1) * 64, cols], rsc[:])

            def deferred():
                for tp in range(4):
                    blk = cc * 4 + tp
                    ps2 = pt.tile([128, 128], BF16, tag="tp", name="tp")
                    nc.tensor.transpose(
                        ps2[0:128, 0:64],
                        RST[b * 64:(b + 1) * 64, blk * 128:(blk + 1) * 128],
                        IDB[b * 64:(b + 1) * 64, b * 64:b * 64 + 64],
                    )
                    nc.vector.tensor_copy(
                        XC2[:, blk * 128 + b * 64:blk * 128 + b * 64 + 64],
                        ps2[0:128, 0:64],
                    )
            return deferred

        pend = []
        for b in range(B2):
            for cc in range(NCH):
                pend.append(d1_chunk(b, cc))
                if len(pend) > 3:
                    pend.pop(0)()
        for fn in pend:
            fn()

        if PHASES < 4:
            return nc
        # ---- B1: x1'_s^T packed = (S_s @ rs)^T ----
        XC3 = pers.tile([128, N], F8, tag="x8", name="XC3", bufs=2)
        product_stream(
            lambda s, b, a2: dr_slice(XC2, a2),
            hop1_sink(lambda s, b: h2[2 * s], lambda s, b: XC3),
            pack_batches=True,
        )

        if PHASES < 5:
            return nc
        # ---- B2: x2'_s^T packed = 2*(S_s @ x1')^T - rs^T ----
        product_stream(
            lambda s, b, a2: dr_slice(XC3, a2),
            hop2_sink(lambda s, b: h2[2 * s + 1], lambda b: RST),
            pack_batches=True,
        )

        if PHASES < 6:
            return nc
        # ---- D2: dense c + tanh + blend + transpose + out ----
        # blend runs at base-64 partitions so the two-input DVE ops read
        # XT/RUT rows 64:128 directly (no alignment copies)
        def d2_chunk(b, cc):
            cols = slice(cc * CH, (cc + 1) * CH)
            ps = pp.tile([128, CH], F32, tag="pp", name="pp")
            pc = ps[0:64, :]
            # batched loads: 4 inputs-half feats (h1 rows 0:64) and 4
            # states-half feats (h2 rows b*64:), reusing x1q slots
            sgi = pers.tile([64, 4 * CH], BF16, tag="x1q", name="sgi",
                            bufs=4)
            nc.sync.dma_start(
                sgi[:].rearrange("p (i c) -> p i c", i=4),
                h1[b][:, 0:64, cols].rearrange("i p c -> p i c"),
            )
            sgs = pers.tile([64, 4 * CH], BF16, tag="x1q", name="sgs",
                            bufs=4)
            nc.sync.dma_start(
                sgs[:].rearrange("p (i c) -> p i c", i=4),
                h2[:, b * 64:(b + 1) * 64, cols].rearrange("i p c -> p i c"),
            )
            nmm = 0
            for m in range(5):
                # inputs-half: lhsT at rows 0:64, rhs at base 0
                if m == 0:
                    rhs_i = XT[b][0:64, cols]
                else:
                    rhs_i = sgi[:, (m - 1) * CH:m * CH]
                nc.tensor.matmul(
                    pc, WC[0:64, m * 64:(m + 1) * 64], rhs_i,
                    start=(nmm == 0), stop=False,
                )
                nmm += 1
                # states-half: stage everything at base 0 so every matmul
                # keeps tile_position (0,0)
                if m == 0:
                    sgr = stage.tile([64, CH], BF16, tag="sgr", name="sgr",
                                     bufs=3)
                    nc.vector.tensor_copy(
                        sgr[:], RST[b * 64:(b + 1) * 64, cols]
                    )
                    rhs_s = sgr[:]
                else:
                    rhs_s = sgs[:, (m - 1) * CH:m * CH]
                lhs_s = WC[0:64, 320 + m * 64:320 + (m + 1) * 64]
                nmm += 1
                nc.tensor.matmul(
                    pc, lhs_s, rhs_s, start=False, stop=(nmm == 10),
                )
            ctf = stage.tile([128, CH], F32, tag="f1", name="f1", bufs=2)
            nc.scalar.activation(ctf[64:128, :], pc, AF.Tanh, bias=BC[:])
            t1 = stage.tile([128, CH], F32, tag="f2", name="f2", bufs=2)
            nc.vector.tensor_sub(t1[64:128, :], XT[b][64:128, cols],
                                 ctf[64:128, :])
            t2 = stage.tile([128, CH], F32, tag="f3", name="f3", bufs=2)
            nc.vector.tensor_mul(t2[64:128, :], t1[64:128, :],
                                 RUT[b][64:128, cols])
            otf = stage.tile([128, CH], F32, tag="f4", name="f4", bufs=3)
            nc.vector.tensor_add(otf[64:128, :], ctf[64:128, :],
                                 t2[64:128, :])

            def deferred():
                ont = onat.tile([128, 4 * 64], BF16, tag="on", name="on")
                for tp in range(4):
                    pso = pt.tile([128, 128], F32, tag="tp", name="tp")
                    nc.tensor.transpose(
                        pso[0:128, 0:64],
                        otf[64:128, tp * 128:(tp + 1) * 128],
                        IDF[64:128, 64:128],
                    )
                    nc.vector.tensor_copy(
                        ont[:, tp * 64:(tp + 1) * 64], pso[0:128, 0:64]
                    )
                nc.sync.dma_start(
                    out_ap[b, cc * CH:(cc + 1) * CH, :].rearrange(
                        "(a p) d -> p a d", p=128),
                    ont[:].rearrange("p (a d) -> p a d", a=4),
                )
            return deferred

        pend = []
        for b in range(B2):
            for cc in range(NCH):
                pend.append(d2_chunk(b, cc))
                if len(pend) > 3:
                    pend.pop(0)()
        for fn in pend:
            fn()

    return nc


def _get_nc():
    if "nc" not in _CACHE:
        nc = _build()
        nc.compile()
        _CACHE["nc"] = nc
    return _CACHE["nc"]


def kernel(inputs, states, supports, W_ru, b_ru, W_c, b_c, _trace=False):
    bf = ml_dtypes.bfloat16
    f8 = ml_dtypes.float8_e4m3
    B = inputs.shape[0]
    ncore = 8
    bper = B // ncore

    x_cat32 = np.concatenate([inputs, states], axis=-1)              # [16,N,128]
    x_cat = x_cat32.astype(bf)
    x_cat8 = x_cat32.astype(f8)
    # blocked S^T fp8 [s, g, k, p, a2, c]: supT[s, m, n] with
    # m = a2*256 + k*128 + p, n = g*GC + c
    supT8 = (np.asarray(supports).transpose(0, 2, 1) * S_SC).astype(f8)
    supB = np.ascontiguousarray(
        supT8.reshape(NSUP, ND, 2, 128, NG, GC).transpose(0, 4, 2, 3, 1, 5))
    wru = np.asarray(W_ru).astype(bf)
    wc = np.asarray(W_c).astype(bf)
    bru = np.asarray(b_ru).astype(np.float32).reshape(2 * H, 1)
    bc = np.asarray(b_c).astype(np.float32).reshape(H, 1)

    nc = _get_nc()
    in_maps = []
    for c in range(ncore):
        in_maps.append({
            "xcat": np.ascontiguousarray(x_cat[c * bper:(c + 1) * bper]),
            "xcat8": np.ascontiguousarray(x_cat8[c * bper:(c + 1) * bper]),
            "supB": supB,
            "wru": wru,
            "wc": wc,
            "bru": bru,
            "bc": bc,
        })
    res = run_bass_kernel_spmd(
        nc, in_maps, core_ids=list(range(ncore)), trace=_trace,
    )
    outs = [r["out"] for r in res.results]
    full = np.concatenate(outs, axis=0).astype(np.float32)           # [16,N,64]
    if _trace:
        kernel.last_results = res
    return full, full
